# revision 7
# baseline (speedup 1.0000x reference)
"""Trainium2 Bass kernel for DeepICD candidate attention.

Reference computation (per batch b):
    S     = X[b] @ a_w                      [L, OS]     (a_b drops out of softmax)
    alpha = softmax(S, axis=L)
    Xp    = alpha^T @ X[b]                  [OS, D]
    Xph   = Xp @ hw_eff + hb_eff            [OS, LAB]   (BN folded into hw/hb on host)
    Xpf   = relu(Xph)
    bLV   = labDescVec[candidate[b]]        [NC, LAB]
    sc    = Xpf @ bLV^T                     [OS, NC]
    a2    = softmax(sc, axis=OS)
    out   = a2^T @ Xpf                      [NC, LAB]

Sharding: data-parallel over batch B=16 across 8 NeuronCores (2 batches/core);
weights and labDescVec replicated.

softmax over L is computed without max subtraction (S ~ N(0,1), |S| < ~6, exp
is safe in fp32) so the L-dim reduction becomes a matmul with a ones vector.
"""

import numpy as np

P = 128
NB = 2          # batches per core
L = 2048
D = 1024
OS = 64
NCC = 256       # candidates per sample
LAB = 1024
CLS = 8921
NT = L // P     # 16 l-tiles
DC = D // P     # 8 d-chunks
HC = LAB // P   # 8 h-chunks
CC = NCC // P   # 2 candidate chunks
N_CORES = 8
BN_EPS = 1e-5

_PROG = None


def _build_program():
    import concourse.bass as bass
    import concourse.bacc as bacc
    import concourse.tile as tile
    from concourse import mybir
    from concourse.masks import make_identity

    f32 = mybir.dt.float32
    bf16 = mybir.dt.bfloat16
    i32 = mybir.dt.int32
    AF = mybir.ActivationFunctionType

    # Bacc (not plain Bass): its compile() pass legalizes multi-wait
    # instructions via event semaphores, which walrus codegen requires.
    nc = bacc.Bacc("TRN2", target_bir_lowering=False, debug=False,
                   num_devices=N_CORES)
    X = nc.dram_tensor("X", [NB, L, D], f32, kind="ExternalInput")
    cand = nc.dram_tensor("cand", [NB, NCC], i32, kind="ExternalInput")
    aw = nc.dram_tensor("aw", [D, OS], f32, kind="ExternalInput")
    hw = nc.dram_tensor("hw", [D, LAB], f32, kind="ExternalInput")
    hb = nc.dram_tensor("hb", [LAB], f32, kind="ExternalInput")
    lab = nc.dram_tensor("lab", [CLS, LAB], f32, kind="ExternalInput")
    out_d = nc.dram_tensor("out", [NB, NCC, LAB], f32, kind="ExternalOutput")

    with tile.TileContext(nc) as tc:
        with (
            tc.tile_pool(name="singles", bufs=1) as singles,
            tc.tile_pool(name="gat", bufs=NB * CC) as gat,
            tc.tile_pool(name="xin", bufs=3) as xin,
            tc.tile_pool(name="work", bufs=2) as work,
            tc.tile_pool(name="outp", bufs=4) as outp,
            tc.tile_pool(name="pa", bufs=2, space="PSUM") as pa,
            tc.tile_pool(name="pb", bufs=3, space="PSUM") as pb,
            tc.tile_pool(name="pacc", bufs=1, space="PSUM") as pacc,
        ):
            # ---- constants / weights ----
            ident = singles.tile([P, P], bf16)
            make_identity(nc, ident[:])
            ones_col = singles.tile([P, 1], bf16)
            nc.vector.memset(ones_col[:], 1.0)

            aw_bf = singles.tile([P, DC, OS], bf16)
            nc.gpsimd.dma_start(
                out=aw_bf[:], in_=aw[:, :].rearrange("(c p) o -> p c o", p=P)
            )
            hw_bf = singles.tile([P, DC, LAB], bf16)
            nc.gpsimd.dma_start(
                out=hw_bf[:], in_=hw[:, :].rearrange("(c p) h -> p c h", p=P)
            )
            hb_sb = singles.tile([P, HC], f32)
            nc.sync.dma_start(
                out=hb_sb[:], in_=hb[:].rearrange("(c p) -> p c", p=P)
            )
            cand_sb = singles.tile([P, NB, CC], i32)
            nc.gpsimd.dma_start(
                out=cand_sb[:], in_=cand[:, :].rearrange("b (c p) -> p b c", p=P)
            )
            # candidate-row gathers issued up-front: no compute deps, and the
            # indirect DMA instruction tolerates very few sync waits
            blv_f = {}
            for b in range(NB):
                for cc in range(CC):
                    bf_t = gat.tile([P, LAB], f32, tag="blvf", name=f"blvf_{b}_{cc}")
                    nc.gpsimd.indirect_dma_start(
                        out=bf_t[:], out_offset=None, in_=lab[:, :],
                        in_offset=bass.IndirectOffsetOnAxis(
                            ap=cand_sb[:, b, cc:cc + 1], axis=0,
                        ),
                    )
                    blv_f[b, cc] = bf_t

            for b in range(NB):
                # ======== phase A: attention pooling over L ========
                xpu = pacc.tile([OS, D], f32, tag="xpu")    # unnormalized alpha^T X
                zz = pacc.tile([OS, 1], f32, tag="zz")      # softmax partition func

                for i in range(NT):
                    xbf = xin.tile([P, D], bf16, tag="xbf")
                    nc.gpsimd.dma_start(
                        out=xbf[:], in_=X[b, i * P:(i + 1) * P, :]
                    )
                    # X tile transposed (d on partitions) for the S matmul
                    xt_sb = work.tile([P, DC, P], bf16, tag="xt")
                    for c in range(DC):
                        tp = pa.tile([P, P], bf16, tag="tp")
                        nc.tensor.transpose(
                            out=tp[:], in_=xbf[:, c * P:(c + 1) * P],
                            identity=ident[:],
                        )
                        nc.vector.tensor_copy(out=xt_sb[:, c, :], in_=tp[:])
                    s_ps = pb.tile([P, OS], f32, tag="mm")
                    for c in range(DC):
                        nc.tensor.matmul(
                            out=s_ps[:], lhsT=xt_sb[:, c, :], rhs=aw_bf[:, c, :],
                            start=(c == 0), stop=(c == DC - 1),
                        )
                    e_sb = work.tile([P, OS], bf16, tag="e")
                    nc.scalar.activation(out=e_sb[:], in_=s_ps[:], func=AF.Exp)
                    for nh in range(2):
                        nc.tensor.matmul(
                            out=xpu[:, nh * 512:(nh + 1) * 512],
                            lhsT=e_sb[:], rhs=xbf[:, nh * 512:(nh + 1) * 512],
                            start=(i == 0), stop=(i == NT - 1),
                            skip_group_check=True,
                        )
                    nc.tensor.matmul(
                        out=zz[:], lhsT=e_sb[:], rhs=ones_col[:],
                        start=(i == 0), stop=(i == NT - 1),
                        skip_group_check=True,
                    )

                # ======== phase B: normalize + project ========
                rz = work.tile([OS, 1], f32, tag="rz")
                nc.vector.reciprocal(out=rz[:], in_=zz[:])
                xp_bf = work.tile([OS, D], bf16, tag="xp")
                nc.vector.tensor_scalar(
                    out=xp_bf[:], in0=xpu[:], scalar1=rz[:], scalar2=None,
                    op0=mybir.AluOpType.mult,
                )
                # Xp^T (d on partitions) for the h-projection
                xpt_sb = work.tile([P, DC, OS], bf16, tag="xpt")
                for c in range(DC):
                    tp2 = pa.tile([P, OS], bf16, tag="tp")
                    nc.tensor.transpose(
                        out=tp2[:], in_=xp_bf[:, c * P:(c + 1) * P],
                        identity=ident[:OS, :OS],
                    )
                    nc.scalar.copy(out=xpt_sb[:, c, :], in_=tp2[:])
                # Xph^T = hw^T Xp^T, per h-chunk; bias+relu on ACT
                xpft_sb = work.tile([P, HC, OS], bf16, tag="xpft")
                for hc in range(HC):
                    ph = pb.tile([P, OS], f32, tag="mm")
                    for c in range(DC):
                        nc.tensor.matmul(
                            out=ph[:],
                            lhsT=hw_bf[:, c, hc * P:(hc + 1) * P],
                            rhs=xpt_sb[:, c, :],
                            start=(c == 0), stop=(c == DC - 1),
                        )
                    nc.scalar.activation(
                        out=xpft_sb[:, hc, :], in_=ph[:], func=AF.Relu,
                        bias=hb_sb[:, hc:hc + 1],
                    )
                # Xpf in [OS, LAB] orientation for the final matmul
                xpf_sb = work.tile([OS, HC, P], bf16, tag="xpf")
                for hc in range(HC):
                    tp3 = pa.tile([OS, P], bf16, tag="tp")
                    nc.tensor.transpose(
                        out=tp3[:], in_=xpft_sb[:, hc, :], identity=ident[:],
                    )
                    nc.vector.tensor_copy(out=xpf_sb[:, hc, :], in_=tp3[:])

                # ======== phase C: candidate gather + attention ========
                blvT = work.tile([P, HC, NCC], bf16, tag="blvT")
                for cc in range(CC):
                    blv_bf = work.tile([P, LAB], bf16, tag="blvb")
                    nc.vector.tensor_copy(out=blv_bf[:], in_=blv_f[b, cc][:])
                    for hc in range(HC):
                        tp4 = pa.tile([P, P], bf16, tag="tp")
                        nc.tensor.transpose(
                            out=tp4[:], in_=blv_bf[:, hc * P:(hc + 1) * P],
                            identity=ident[:],
                        )
                        nc.scalar.copy(
                            out=blvT[:, hc, cc * P:(cc + 1) * P], in_=tp4[:]
                        )
                a2t_sb = work.tile([OS, CC, P], bf16, tag="a2t")
                for cc in range(CC):
                    s2 = pb.tile([P, OS], f32, tag="mm")
                    for hc in range(HC):
                        nc.tensor.matmul(
                            out=s2[:],
                            lhsT=blvT[:, hc, cc * P:(cc + 1) * P],
                            rhs=xpft_sb[:, hc, :],
                            start=(hc == 0), stop=(hc == HC - 1),
                        )
                    negm = work.tile([P, 1], f32, tag="negm")
                    nc.vector.tensor_reduce(
                        out=negm[:], in_=s2[:], axis=mybir.AxisListType.X,
                        op=mybir.AluOpType.max, negate=True,
                    )
                    e2 = work.tile([P, OS], bf16, tag="e2")
                    sume = work.tile([P, 1], f32, tag="sume")
                    nc.scalar.activation(
                        out=e2[:], in_=s2[:], func=AF.Exp, bias=negm[:],
                        accum_out=sume[:],
                    )
                    rz2 = work.tile([P, 1], f32, tag="rz2")
                    nc.vector.reciprocal(out=rz2[:], in_=sume[:])
                    a2 = work.tile([P, OS], bf16, tag="a2")
                    nc.vector.tensor_scalar(
                        out=a2[:], in0=e2[:], scalar1=rz2[:], scalar2=None,
                        op0=mybir.AluOpType.mult,
                    )
                    tp5 = pa.tile([OS, P], bf16, tag="tp")
                    nc.tensor.transpose(out=tp5[:], in_=a2[:], identity=ident[:])
                    nc.vector.tensor_copy(out=a2t_sb[:, cc, :], in_=tp5[:])

                # ======== phase D: out = a2^T Xpf ========
                for cc in range(CC):
                    for nh in range(2):
                        op = pb.tile([P, 512], f32, tag="mm")
                        nc.tensor.matmul(
                            out=op[:], lhsT=a2t_sb[:, cc, :],
                            rhs=xpf_sb[:, nh * 4:(nh + 1) * 4, :],
                            start=True, stop=True,
                        )
                        ob = outp.tile([P, 512], f32, tag="ob")
                        nc.scalar.copy(out=ob[:], in_=op[:])
                        nc.sync.dma_start(
                            out=out_d[b, cc * P:(cc + 1) * P,
                                      nh * 512:(nh + 1) * 512],
                            in_=ob[:],
                        )
    nc.finalize()
    return nc


def _get_program():
    global _PROG
    if _PROG is None:
        _PROG = _build_program()
    return _PROG


def _make_in_maps(inputs):
    X = np.ascontiguousarray(np.asarray(inputs["X"], dtype=np.float32))
    cand = np.ascontiguousarray(
        np.asarray(inputs["candidate"]).astype(np.int32)
    )
    a_w = np.asarray(inputs["a_w"], dtype=np.float32)
    h_w = np.asarray(inputs["h_w"], dtype=np.float32)
    h_b = np.asarray(inputs["h_b"], dtype=np.float32)
    g = np.asarray(inputs["bn_gamma"], dtype=np.float32)
    be = np.asarray(inputs["bn_beta"], dtype=np.float32)
    mu = np.asarray(inputs["bn_mean"], dtype=np.float32)
    var = np.asarray(inputs["bn_var"], dtype=np.float32)
    lab = np.ascontiguousarray(np.asarray(inputs["labDescVec"], dtype=np.float32))

    s = g / np.sqrt(var + BN_EPS)
    hw_eff = np.ascontiguousarray((h_w * s[None, :]).astype(np.float32))
    hb_eff = ((h_b - mu) * s + be).astype(np.float32)

    in_maps = []
    for ci in range(N_CORES):
        in_maps.append({
            "X": X[ci * NB:(ci + 1) * NB],
            "cand": cand[ci * NB:(ci + 1) * NB],
            "aw": a_w,
            "hw": hw_eff,
            "hb": hb_eff,
            "lab": lab,
        })
    return in_maps


def run(inputs, trace=False, tmpdir=None):
    from concourse.bass_utils import run_bass_kernel_spmd

    nc = _get_program()
    in_maps = _make_in_maps(inputs)
    kwargs = {}
    if trace and tmpdir is None:
        tmpdir = "/root/problem/trace_out"
        import os
        import shutil

        shutil.rmtree(tmpdir, ignore_errors=True)
        os.makedirs(tmpdir, exist_ok=True)
    if tmpdir is not None:
        kwargs["tmpdir"] = tmpdir
    res = run_bass_kernel_spmd(
        nc, in_maps, list(range(N_CORES)), trace=trace, **kwargs,
    )
    out = np.concatenate([r["out"] for r in res.results], axis=0)
    return out, res


def kernel(**inputs):
    out, _ = run(inputs, trace=False)
    return out


# revision 12
# speedup vs baseline: 1.0895x; 1.0895x over previous
"""Trainium2 Bass kernel for DeepICD candidate attention.

Reference computation (per batch b):
    S     = X[b] @ a_w                      [L, OS]     (a_b drops out of softmax)
    alpha = softmax(S, axis=L)
    Xp    = alpha^T @ X[b]                  [OS, D]
    Xph   = Xp @ hw_eff + hb_eff            [OS, LAB]   (BN folded into hw/hb on host)
    Xpf   = relu(Xph)
    bLV   = labDescVec[candidate[b]]        [NC, LAB]
    sc    = Xpf @ bLV^T                     [OS, NC]
    a2    = softmax(sc, axis=OS)
    out   = a2^T @ Xpf                      [NC, LAB]

Sharding: data-parallel over batch B=16 across 8 NeuronCores (2 batches/core);
weights and labDescVec replicated.

softmax over L is computed without max subtraction (S ~ N(0,1), |S| < ~6, exp
is safe in fp32) so the L-dim reduction becomes a matmul with a ones vector.
"""

import numpy as np

P = 128
NB = 2          # batches per core
L = 2048
D = 1024
OS = 64
NCC = 256       # candidates per sample
LAB = 1024
CLS = 8921
NT = L // P     # 16 l-tiles
DC = D // P     # 8 d-chunks
HC = LAB // P   # 8 h-chunks
CC = NCC // P   # 2 candidate chunks
N_CORES = 8
BN_EPS = 1e-5

_PROG = None


def _build_program():
    import concourse.bass as bass
    import concourse.bacc as bacc
    import concourse.tile as tile
    from concourse import mybir
    from concourse.masks import make_identity

    f32 = mybir.dt.float32
    bf16 = mybir.dt.bfloat16
    i32 = mybir.dt.int32
    AF = mybir.ActivationFunctionType

    # Bacc (not plain Bass): its compile() pass legalizes multi-wait
    # instructions via event semaphores, which walrus codegen requires.
    nc = bacc.Bacc("TRN2", target_bir_lowering=False, debug=False,
                   num_devices=N_CORES)
    X = nc.dram_tensor("X", [NB, L, D], f32, kind="ExternalInput")
    cand = nc.dram_tensor("cand", [NB, NCC], i32, kind="ExternalInput")
    aw = nc.dram_tensor("aw", [D, OS], f32, kind="ExternalInput")
    hw = nc.dram_tensor("hw", [D, LAB], f32, kind="ExternalInput")
    hb = nc.dram_tensor("hb", [LAB], f32, kind="ExternalInput")
    lab = nc.dram_tensor("lab", [CLS, LAB], f32, kind="ExternalInput")
    out_d = nc.dram_tensor("out", [NB, NCC, LAB], f32, kind="ExternalOutput")

    with tile.TileContext(nc) as tc:
        with (
            tc.tile_pool(name="singles", bufs=1) as singles,
            tc.tile_pool(name="gat", bufs=NB * CC) as gat,
            tc.tile_pool(name="xin", bufs=3) as xin,
            tc.tile_pool(name="work", bufs=2) as work,
            tc.tile_pool(name="outp", bufs=4) as outp,
            tc.tile_pool(name="pa", bufs=2, space="PSUM") as pa,
            tc.tile_pool(name="pb", bufs=3, space="PSUM") as pb,
            tc.tile_pool(name="pacc", bufs=1, space="PSUM") as pacc,
        ):
            # ---- constants / weights ----
            ident = singles.tile([P, P], bf16)
            make_identity(nc, ident[:])
            ones_col = singles.tile([P, 1], bf16)
            nc.vector.memset(ones_col[:], 1.0)

            aw_bf = singles.tile([P, DC, OS], bf16)
            nc.gpsimd.dma_start(
                out=aw_bf[:], in_=aw[:, :].rearrange("(c p) o -> p c o", p=P)
            )
            hw_bf = singles.tile([P, DC, LAB], bf16)
            nc.gpsimd.dma_start(
                out=hw_bf[:], in_=hw[:, :].rearrange("(c p) h -> p c h", p=P)
            )
            hb_sb = singles.tile([P, HC], f32)
            nc.sync.dma_start(
                out=hb_sb[:], in_=hb[:].rearrange("(c p) -> p c", p=P)
            )
            cand_sb = singles.tile([P, NB, CC], i32)
            nc.gpsimd.dma_start(
                out=cand_sb[:], in_=cand[:, :].rearrange("b (c p) -> p b c", p=P)
            )
            # candidate-row gathers issued up-front: no compute deps, and the
            # indirect DMA instruction tolerates very few sync waits
            blv_f = {}
            for b in range(NB):
                for cc in range(CC):
                    bf_t = gat.tile([P, LAB], f32, tag="blvf", name=f"blvf_{b}_{cc}")
                    nc.gpsimd.indirect_dma_start(
                        out=bf_t[:], out_offset=None, in_=lab[:, :],
                        in_offset=bass.IndirectOffsetOnAxis(
                            ap=cand_sb[:, b, cc:cc + 1], axis=0,
                        ),
                    )
                    blv_f[b, cc] = bf_t

            for b in range(NB):
                # ======== phase A: attention pooling over L ========
                xpu = pacc.tile([OS, D], f32, tag="xpu")    # unnormalized alpha^T X
                zz = pacc.tile([OS, 1], f32, tag="zz")      # softmax partition func

                for i in range(NT):
                    xbf = xin.tile([P, D], bf16, tag="xbf")
                    nc.gpsimd.dma_start(
                        out=xbf[:], in_=X[b, i * P:(i + 1) * P, :]
                    )
                    # X tile transposed (d on partitions) for the S matmul.
                    # All 8 chunk transposes land in one PSUM bank, evacuated
                    # with a single copy (per-chunk copies serialize PE vs DVE)
                    xt_sb = work.tile([P, DC, P], bf16, tag="xt")
                    tp = pa.tile([P, DC, P], bf16, tag="tp")
                    for c in range(DC):
                        nc.tensor.transpose(
                            out=tp[:, c, :], in_=xbf[:, c * P:(c + 1) * P],
                            identity=ident[:],
                        )
                    if i % 2 == 0:
                        nc.vector.tensor_copy(out=xt_sb[:], in_=tp[:])
                    else:
                        nc.scalar.copy(out=xt_sb[:], in_=tp[:])
                    s_ps = pb.tile([P, OS], f32, tag="mm")
                    for c in range(DC):
                        nc.tensor.matmul(
                            out=s_ps[:], lhsT=xt_sb[:, c, :], rhs=aw_bf[:, c, :],
                            start=(c == 0), stop=(c == DC - 1),
                        )
                    e_sb = work.tile([P, OS], bf16, tag="e")
                    nc.scalar.activation(out=e_sb[:], in_=s_ps[:], func=AF.Exp)
                    for nh in range(2):
                        nc.tensor.matmul(
                            out=xpu[:, nh * 512:(nh + 1) * 512],
                            lhsT=e_sb[:], rhs=xbf[:, nh * 512:(nh + 1) * 512],
                            start=(i == 0), stop=(i == NT - 1),
                            skip_group_check=True,
                        )
                    nc.tensor.matmul(
                        out=zz[:], lhsT=e_sb[:], rhs=ones_col[:],
                        start=(i == 0), stop=(i == NT - 1),
                        skip_group_check=True,
                    )

                # ======== phase B: normalize + project ========
                rz = work.tile([OS, 1], f32, tag="rz")
                nc.vector.reciprocal(out=rz[:], in_=zz[:])
                xp_bf = work.tile([OS, D], bf16, tag="xp")
                nc.vector.tensor_scalar(
                    out=xp_bf[:], in0=xpu[:], scalar1=rz[:], scalar2=None,
                    op0=mybir.AluOpType.mult,
                )
                # Xp^T (d on partitions) for the h-projection
                xpt_sb = work.tile([P, DC, OS], bf16, tag="xpt")
                tp2 = pa.tile([P, DC, OS], bf16, tag="tp")
                for c in range(DC):
                    nc.tensor.transpose(
                        out=tp2[:, c, :], in_=xp_bf[:, c * P:(c + 1) * P],
                        identity=ident[:OS, :OS],
                    )
                nc.scalar.copy(out=xpt_sb[:], in_=tp2[:])
                # Xph^T = hw^T Xp^T, per h-chunk; bias+relu on ACT
                xpft_sb = work.tile([P, HC, OS], bf16, tag="xpft")
                for hc in range(HC):
                    ph = pb.tile([P, OS], f32, tag="mm")
                    for c in range(DC):
                        nc.tensor.matmul(
                            out=ph[:],
                            lhsT=hw_bf[:, c, hc * P:(hc + 1) * P],
                            rhs=xpt_sb[:, c, :],
                            start=(c == 0), stop=(c == DC - 1),
                        )
                    nc.scalar.activation(
                        out=xpft_sb[:, hc, :], in_=ph[:], func=AF.Relu,
                        bias=hb_sb[:, hc:hc + 1],
                    )
                # Xpf in [OS, LAB] orientation for the final matmul
                xpf_sb = work.tile([OS, HC, P], bf16, tag="xpf")
                tp3 = pa.tile([OS, HC, P], bf16, tag="tp")
                for hc in range(HC):
                    nc.tensor.transpose(
                        out=tp3[:, hc, :], in_=xpft_sb[:, hc, :],
                        identity=ident[:],
                    )
                nc.vector.tensor_copy(out=xpf_sb[:], in_=tp3[:])

                # ======== phase C: candidate gather + attention ========
                blvT = work.tile([P, HC, NCC], bf16, tag="blvT")
                for cc in range(CC):
                    blv_bf = work.tile([P, LAB], bf16, tag="blvb")
                    nc.vector.tensor_copy(out=blv_bf[:], in_=blv_f[b, cc][:])
                    tp4 = pa.tile([P, HC, P], bf16, tag="tp")
                    for hc in range(HC):
                        nc.tensor.transpose(
                            out=tp4[:, hc, :], in_=blv_bf[:, hc * P:(hc + 1) * P],
                            identity=ident[:],
                        )
                    nc.scalar.copy(
                        out=blvT[:, :, cc * P:(cc + 1) * P], in_=tp4[:]
                    )
                a2t_sb = work.tile([OS, CC, P], bf16, tag="a2t")
                for cc in range(CC):
                    s2 = pb.tile([P, OS], f32, tag="mm")
                    for hc in range(HC):
                        nc.tensor.matmul(
                            out=s2[:],
                            lhsT=blvT[:, hc, cc * P:(cc + 1) * P],
                            rhs=xpft_sb[:, hc, :],
                            start=(hc == 0), stop=(hc == HC - 1),
                        )
                    negm = work.tile([P, 1], f32, tag="negm")
                    nc.vector.tensor_reduce(
                        out=negm[:], in_=s2[:], axis=mybir.AxisListType.X,
                        op=mybir.AluOpType.max, negate=True,
                    )
                    e2 = work.tile([P, OS], bf16, tag="e2")
                    sume = work.tile([P, 1], f32, tag="sume")
                    nc.scalar.activation(
                        out=e2[:], in_=s2[:], func=AF.Exp, bias=negm[:],
                        accum_out=sume[:],
                    )
                    rz2 = work.tile([P, 1], f32, tag="rz2")
                    nc.vector.reciprocal(out=rz2[:], in_=sume[:])
                    a2 = work.tile([P, OS], bf16, tag="a2")
                    nc.vector.tensor_scalar(
                        out=a2[:], in0=e2[:], scalar1=rz2[:], scalar2=None,
                        op0=mybir.AluOpType.mult,
                    )
                    tp5 = pa.tile([OS, P], bf16, tag="tp")
                    nc.tensor.transpose(out=tp5[:], in_=a2[:], identity=ident[:])
                    nc.vector.tensor_copy(out=a2t_sb[:, cc, :], in_=tp5[:])

                # ======== phase D: out = a2^T Xpf ========
                for cc in range(CC):
                    for nh in range(2):
                        op = pb.tile([P, 512], f32, tag="mm")
                        nc.tensor.matmul(
                            out=op[:], lhsT=a2t_sb[:, cc, :],
                            rhs=xpf_sb[:, nh * 4:(nh + 1) * 4, :],
                            start=True, stop=True,
                        )
                        ob = outp.tile([P, 512], f32, tag="ob")
                        nc.scalar.copy(out=ob[:], in_=op[:])
                        nc.sync.dma_start(
                            out=out_d[b, cc * P:(cc + 1) * P,
                                      nh * 512:(nh + 1) * 512],
                            in_=ob[:],
                        )
    nc.finalize()
    return nc


def _get_program():
    global _PROG
    if _PROG is None:
        _PROG = _build_program()
    return _PROG


def _make_in_maps(inputs):
    X = np.ascontiguousarray(np.asarray(inputs["X"], dtype=np.float32))
    cand = np.ascontiguousarray(
        np.asarray(inputs["candidate"]).astype(np.int32)
    )
    a_w = np.asarray(inputs["a_w"], dtype=np.float32)
    h_w = np.asarray(inputs["h_w"], dtype=np.float32)
    h_b = np.asarray(inputs["h_b"], dtype=np.float32)
    g = np.asarray(inputs["bn_gamma"], dtype=np.float32)
    be = np.asarray(inputs["bn_beta"], dtype=np.float32)
    mu = np.asarray(inputs["bn_mean"], dtype=np.float32)
    var = np.asarray(inputs["bn_var"], dtype=np.float32)
    lab = np.ascontiguousarray(np.asarray(inputs["labDescVec"], dtype=np.float32))

    s = g / np.sqrt(var + BN_EPS)
    hw_eff = np.ascontiguousarray((h_w * s[None, :]).astype(np.float32))
    hb_eff = ((h_b - mu) * s + be).astype(np.float32)

    in_maps = []
    for ci in range(N_CORES):
        in_maps.append({
            "X": X[ci * NB:(ci + 1) * NB],
            "cand": cand[ci * NB:(ci + 1) * NB],
            "aw": a_w,
            "hw": hw_eff,
            "hb": hb_eff,
            "lab": lab,
        })
    return in_maps


def run(inputs, trace=False, tmpdir=None):
    from concourse.bass_utils import run_bass_kernel_spmd

    nc = _get_program()
    in_maps = _make_in_maps(inputs)
    kwargs = {}
    if trace and tmpdir is None:
        tmpdir = "/root/problem/trace_out"
        import os
        import shutil

        shutil.rmtree(tmpdir, ignore_errors=True)
        os.makedirs(tmpdir, exist_ok=True)
    if tmpdir is not None:
        kwargs["tmpdir"] = tmpdir
    res = run_bass_kernel_spmd(
        nc, in_maps, list(range(N_CORES)), trace=trace, **kwargs,
    )
    out = np.concatenate([r["out"] for r in res.results], axis=0)
    return out, res


def kernel(**inputs):
    out, _ = run(inputs, trace=False)
    return out


# revision 19
# speedup vs baseline: 1.3099x; 1.2023x over previous
"""Trainium2 Bass kernel for DeepICD candidate attention.

Reference computation (per batch b):
    S     = X[b] @ a_w                      [L, OS]     (a_b drops out of softmax)
    alpha = softmax(S, axis=L)
    Xp    = alpha^T @ X[b]                  [OS, D]
    Xph   = Xp @ hw_eff + hb_eff            [OS, LAB]   (BN folded into hw/hb on host)
    Xpf   = relu(Xph)
    bLV   = labDescVec[candidate[b]]        [NC, LAB]
    sc    = Xpf @ bLV^T                     [OS, NC]
    a2    = softmax(sc, axis=OS)
    out   = a2^T @ Xpf                      [NC, LAB]

Sharding: data-parallel over batch B=16 across 8 NeuronCores (2 batches/core);
weights and labDescVec replicated.

softmax over L is computed without max subtraction (S ~ N(0,1), |S| < ~6, exp
is safe in fp32) so the L-dim reduction becomes a matmul with a ones vector.
"""

import numpy as np

P = 128
NB = 2          # batches per core
L = 2048
D = 1024
OS = 64
NCC = 256       # candidates per sample
LAB = 1024
CLS = 8921
NT = L // P     # 16 l-tiles
DC = D // P     # 8 d-chunks
HC = LAB // P   # 8 h-chunks
CC = NCC // P   # 2 candidate chunks
N_CORES = 8
BN_EPS = 1e-5

_PROG = None


def _build_program():
    import concourse.bass as bass
    import concourse.bacc as bacc
    import concourse.tile as tile
    from concourse import mybir
    from concourse.masks import make_identity

    f32 = mybir.dt.float32
    bf16 = mybir.dt.bfloat16
    i32 = mybir.dt.int32
    AF = mybir.ActivationFunctionType

    # Bacc (not plain Bass): its compile() pass legalizes multi-wait
    # instructions via event semaphores, which walrus codegen requires.
    nc = bacc.Bacc("TRN2", target_bir_lowering=False, debug=False,
                   num_devices=N_CORES)
    X = nc.dram_tensor("X", [NB, L, D], f32, kind="ExternalInput")
    cand = nc.dram_tensor("cand", [NB, NCC], i32, kind="ExternalInput")
    aw = nc.dram_tensor("aw", [D, OS], f32, kind="ExternalInput")
    hw = nc.dram_tensor("hw", [D, LAB], f32, kind="ExternalInput")
    hb = nc.dram_tensor("hb", [LAB], f32, kind="ExternalInput")
    lab = nc.dram_tensor("lab", [CLS, LAB], f32, kind="ExternalInput")
    out_d = nc.dram_tensor("out", [NB, NCC, LAB], f32, kind="ExternalOutput")

    with tile.TileContext(nc) as tc:
        with (
            tc.tile_pool(name="singles", bufs=1) as singles,
            tc.tile_pool(name="gat", bufs=NB * CC) as gat,
            tc.tile_pool(name="xin", bufs=3) as xin,
            tc.tile_pool(name="work", bufs=2) as work,
            tc.tile_pool(name="outp", bufs=4) as outp,
            tc.tile_pool(name="pa", bufs=2, space="PSUM") as pa,
            tc.tile_pool(name="pb", bufs=3, space="PSUM") as pb,
            tc.tile_pool(name="pacc", bufs=1, space="PSUM") as pacc,
        ):
            # ---- constants / weights ----
            ident = singles.tile([P, P], bf16)
            make_identity(nc, ident[:])
            ones_col = singles.tile([P, 1], bf16)
            nc.vector.memset(ones_col[:], 1.0)
            ones_row = singles.tile([1, OS], bf16)
            nc.vector.memset(ones_row[:], 1.0)
            hb_bf = singles.tile([1, LAB], bf16)
            nc.gpsimd.dma_start(out=hb_bf[:], in_=hb[None, :])

            aw_bf = singles.tile([P, DC, OS], bf16)
            nc.gpsimd.dma_start(
                out=aw_bf[:], in_=aw[:, :].rearrange("(c p) o -> p c o", p=P)
            )
            hw_bf = singles.tile([P, DC, LAB], bf16)
            nc.gpsimd.dma_start(
                out=hw_bf[:], in_=hw[:, :].rearrange("(c p) h -> p c h", p=P)
            )
            cand_sb = singles.tile([P, NB, CC], i32)
            nc.gpsimd.dma_start(
                out=cand_sb[:], in_=cand[:, :].rearrange("b (c p) -> p b c", p=P)
            )
            # candidate-row gathers issued up-front: no compute deps, and the
            # indirect DMA instruction tolerates very few sync waits
            blv_f = {}
            for b in range(NB):
                for cc in range(CC):
                    bf_t = gat.tile([P, LAB], f32, tag="blvf", name=f"blvf_{b}_{cc}")
                    nc.gpsimd.indirect_dma_start(
                        out=bf_t[:], out_offset=None, in_=lab[:, :],
                        in_offset=bass.IndirectOffsetOnAxis(
                            ap=cand_sb[:, b, cc:cc + 1], axis=0,
                        ),
                    )
                    blv_f[b, cc] = bf_t

            for b in range(NB):
                # ======== phase A: attention pooling over L ========
                xpu = pacc.tile([OS, D], f32, tag="xpu")    # unnormalized alpha^T X
                zz = pacc.tile([OS, 1], f32, tag="zz")      # softmax partition func

                for ii in range(NT // 2):
                    # two l-tiles per DMA halves SWDGE descriptor-gen cost
                    xbf = xin.tile([P, 2, D], bf16, tag="xbf")
                    nc.gpsimd.dma_start(
                        out=xbf[:],
                        in_=X[b, ii * 2 * P:(ii + 1) * 2 * P, :].rearrange(
                            "(t p) d -> p t d", p=P
                        ),
                    )
                    for t in range(2):
                        i = ii * 2 + t
                        # X tile transposed (d on partitions) for the S matmul.
                        # All 8 chunk transposes land in one PSUM bank,
                        # evacuated with a single copy (per-chunk copies
                        # serialize PE against DVE)
                        xt_sb = work.tile([P, DC, P], bf16, tag="xt")
                        tp = pa.tile([P, DC, P], bf16, tag="tp")
                        for c in range(DC):
                            nc.tensor.transpose(
                                out=tp[:, c, :],
                                in_=xbf[:, t, c * P:(c + 1) * P],
                                identity=ident[:],
                            )
                        if i % 3 == 2:
                            nc.scalar.copy(out=xt_sb[:], in_=tp[:])
                        else:
                            nc.vector.tensor_copy(out=xt_sb[:], in_=tp[:])
                        s_ps = pb.tile([P, OS], f32, tag="mm")
                        for c in range(DC):
                            nc.tensor.matmul(
                                out=s_ps[:], lhsT=xt_sb[:, c, :],
                                rhs=aw_bf[:, c, :],
                                start=(c == 0), stop=(c == DC - 1),
                            )
                        e_sb = work.tile([P, OS], bf16, tag="e")
                        nc.scalar.activation(
                            out=e_sb[:], in_=s_ps[:], func=AF.Exp
                        )
                        for nh in range(2):
                            nc.tensor.matmul(
                                out=xpu[:, nh * 512:(nh + 1) * 512],
                                lhsT=e_sb[:],
                                rhs=xbf[:, t, nh * 512:(nh + 1) * 512],
                                start=(i == 0), stop=(i == NT - 1),
                                skip_group_check=True,
                            )
                        nc.tensor.matmul(
                            out=zz[:], lhsT=e_sb[:], rhs=ones_col[:],
                            start=(i == 0), stop=(i == NT - 1),
                            skip_group_check=True,
                        )

                # ======== phase B: normalize + project ========
                rz = work.tile([OS, 1], f32, tag="rz")
                nc.vector.reciprocal(out=rz[:], in_=zz[:])
                xp_bf = work.tile([OS, D], bf16, tag="xp")
                nc.vector.tensor_scalar(
                    out=xp_bf[:], in0=xpu[:], scalar1=rz[:], scalar2=None,
                    op0=mybir.AluOpType.mult,
                )
                # Xp^T (d on partitions) for the h-projection
                xpt_sb = work.tile([P, DC, OS], bf16, tag="xpt")
                tp2 = pa.tile([P, DC, OS], bf16, tag="tp")
                for c in range(DC):
                    nc.tensor.transpose(
                        out=tp2[:, c, :], in_=xp_bf[:, c * P:(c + 1) * P],
                        identity=ident[:OS, :OS],
                    )
                nc.scalar.copy(out=xpt_sb[:], in_=tp2[:])
                # Xpf = relu(Xp @ hw + hb) in natural [OS, LAB] layout; the
                # hb bias rides the PSUM accumulation as a rank-1 matmul
                xpf_sb = work.tile([OS, LAB], bf16, tag="xpf")
                for nh in range(2):
                    xph = pb.tile([OS, 512], f32, tag="mm")
                    for c in range(DC):
                        nc.tensor.matmul(
                            out=xph[:], lhsT=xpt_sb[:, c, :],
                            rhs=hw_bf[:, c, nh * 512:(nh + 1) * 512],
                            start=(c == 0), stop=False,
                        )
                    nc.tensor.matmul(
                        out=xph[:], lhsT=ones_row[:],
                        rhs=hb_bf[:, nh * 512:(nh + 1) * 512],
                        start=False, stop=True,
                    )
                    nc.scalar.activation(
                        out=xpf_sb[:, nh * 512:(nh + 1) * 512], in_=xph[:],
                        func=AF.Relu,
                    )
                # Xpf^T (h on partitions) for the candidate scores
                xpft_sb = work.tile([P, HC, OS], bf16, tag="xpft")
                tp3 = pa.tile([P, HC, OS], bf16, tag="tp")
                for hc in range(HC):
                    nc.tensor.transpose(
                        out=tp3[:, hc, :], in_=xpf_sb[:, hc * P:(hc + 1) * P],
                        identity=ident[:OS, :OS],
                    )
                nc.vector.tensor_copy(out=xpft_sb[:], in_=tp3[:])

                # ======== phase C: candidate gather + attention ========
                blvT = work.tile([P, HC, NCC], bf16, tag="blvT")
                for cc in range(CC):
                    blv_bf = work.tile([P, LAB], bf16, tag="blvb")
                    nc.vector.tensor_copy(out=blv_bf[:], in_=blv_f[b, cc][:])
                    tp4 = pa.tile([P, HC, P], bf16, tag="tp")
                    for hc in range(HC):
                        nc.tensor.transpose(
                            out=tp4[:, hc, :], in_=blv_bf[:, hc * P:(hc + 1) * P],
                            identity=ident[:],
                        )
                    nc.vector.tensor_copy(
                        out=blvT[:, :, cc * P:(cc + 1) * P], in_=tp4[:]
                    )
                a2t_sb = work.tile([OS, CC, P], bf16, tag="a2t")
                for cc in range(CC):
                    s2 = pb.tile([P, OS], f32, tag="mm")
                    for hc in range(HC):
                        nc.tensor.matmul(
                            out=s2[:],
                            lhsT=blvT[:, hc, cc * P:(cc + 1) * P],
                            rhs=xpft_sb[:, hc, :],
                            start=(hc == 0), stop=(hc == HC - 1),
                        )
                    negm = work.tile([P, 1], f32, tag="negm")
                    nc.vector.tensor_reduce(
                        out=negm[:], in_=s2[:], axis=mybir.AxisListType.X,
                        op=mybir.AluOpType.max, negate=True,
                    )
                    e2 = work.tile([P, OS], bf16, tag="e2")
                    sume = work.tile([P, 1], f32, tag="sume")
                    nc.scalar.activation(
                        out=e2[:], in_=s2[:], func=AF.Exp, bias=negm[:],
                        accum_out=sume[:],
                    )
                    rz2 = work.tile([P, 1], f32, tag="rz2")
                    nc.vector.reciprocal(out=rz2[:], in_=sume[:])
                    a2 = work.tile([P, OS], bf16, tag="a2")
                    nc.vector.tensor_scalar(
                        out=a2[:], in0=e2[:], scalar1=rz2[:], scalar2=None,
                        op0=mybir.AluOpType.mult,
                    )
                    tp5 = pa.tile([OS, P], bf16, tag="tp")
                    nc.tensor.transpose(out=tp5[:], in_=a2[:], identity=ident[:])
                    nc.vector.tensor_copy(out=a2t_sb[:, cc, :], in_=tp5[:])

                # ======== phase D: out = a2^T Xpf ========
                for cc in range(CC):
                    for nh in range(2):
                        op = pb.tile([P, 512], f32, tag="mm")
                        nc.tensor.matmul(
                            out=op[:], lhsT=a2t_sb[:, cc, :],
                            rhs=xpf_sb[:, nh * 512:(nh + 1) * 512],
                            start=True, stop=True,
                        )
                        ob = outp.tile([P, 512], f32, tag="ob")
                        if nh == 0:
                            nc.scalar.copy(out=ob[:], in_=op[:])
                        else:
                            nc.vector.tensor_copy(out=ob[:], in_=op[:])
                        nc.sync.dma_start(
                            out=out_d[b, cc * P:(cc + 1) * P,
                                      nh * 512:(nh + 1) * 512],
                            in_=ob[:],
                        )
    nc.finalize()
    return nc


def _get_program():
    global _PROG
    if _PROG is None:
        _PROG = _build_program()
    return _PROG


def _make_in_maps(inputs):
    X = np.ascontiguousarray(np.asarray(inputs["X"], dtype=np.float32))
    cand = np.ascontiguousarray(
        np.asarray(inputs["candidate"]).astype(np.int32)
    )
    a_w = np.asarray(inputs["a_w"], dtype=np.float32)
    h_w = np.asarray(inputs["h_w"], dtype=np.float32)
    h_b = np.asarray(inputs["h_b"], dtype=np.float32)
    g = np.asarray(inputs["bn_gamma"], dtype=np.float32)
    be = np.asarray(inputs["bn_beta"], dtype=np.float32)
    mu = np.asarray(inputs["bn_mean"], dtype=np.float32)
    var = np.asarray(inputs["bn_var"], dtype=np.float32)
    lab = np.ascontiguousarray(np.asarray(inputs["labDescVec"], dtype=np.float32))

    s = g / np.sqrt(var + BN_EPS)
    hw_eff = np.ascontiguousarray((h_w * s[None, :]).astype(np.float32))
    hb_eff = ((h_b - mu) * s + be).astype(np.float32)

    in_maps = []
    for ci in range(N_CORES):
        in_maps.append({
            "X": X[ci * NB:(ci + 1) * NB],
            "cand": cand[ci * NB:(ci + 1) * NB],
            "aw": a_w,
            "hw": hw_eff,
            "hb": hb_eff,
            "lab": lab,
        })
    return in_maps


def run(inputs, trace=False, tmpdir=None):
    from concourse.bass_utils import run_bass_kernel_spmd

    nc = _get_program()
    in_maps = _make_in_maps(inputs)
    kwargs = {}
    if trace and tmpdir is None:
        tmpdir = "/root/problem/trace_out"
        import os
        import shutil

        shutil.rmtree(tmpdir, ignore_errors=True)
        os.makedirs(tmpdir, exist_ok=True)
    if tmpdir is not None:
        kwargs["tmpdir"] = tmpdir
    res = run_bass_kernel_spmd(
        nc, in_maps, list(range(N_CORES)), trace=trace, **kwargs,
    )
    out = np.concatenate([r["out"] for r in res.results], axis=0)
    return out, res


def kernel(**inputs):
    out, _ = run(inputs, trace=False)
    return out


# revision 25
# speedup vs baseline: 1.6413x; 1.2530x over previous
"""Trainium2 Bass kernel for DeepICD candidate attention.

Reference computation (per batch b):
    S     = X[b] @ a_w                      [L, OS]     (a_b drops out of softmax)
    alpha = softmax(S, axis=L)
    Xp    = alpha^T @ X[b]                  [OS, D]
    Xph   = Xp @ hw_eff + hb_eff            [OS, LAB]   (BN folded into hw/hb on host)
    Xpf   = relu(Xph)
    bLV   = labDescVec[candidate[b]]        [NC, LAB]
    sc    = Xpf @ bLV^T                     [OS, NC]
    a2    = softmax(sc, axis=OS)
    out   = a2^T @ Xpf                      [NC, LAB]

Sharding: data-parallel over batch B=16 across 8 NeuronCores (2 batches/core);
weights and labDescVec replicated.

softmax over L is computed without max subtraction (S ~ N(0,1), |S| < ~6, exp
is safe in fp32) so the L-dim reduction becomes a matmul with a ones vector.
"""

import numpy as np

P = 128
NB = 2          # batches per core
L = 2048
D = 1024
OS = 64
NCC = 256       # candidates per sample
LAB = 1024
CLS = 8921
NT = L // P     # 16 l-tiles
DC = D // P     # 8 d-chunks
HC = LAB // P   # 8 h-chunks
CC = NCC // P   # 2 candidate chunks
N_CORES = 8
BN_EPS = 1e-5

_PROG = None


def _build_program():
    import concourse.bass as bass
    import concourse.bacc as bacc
    import concourse.tile as tile
    from concourse import mybir
    from concourse.masks import make_identity

    f32 = mybir.dt.float32
    bf16 = mybir.dt.bfloat16
    i32 = mybir.dt.int32
    AF = mybir.ActivationFunctionType

    # Bacc (not plain Bass): its compile() pass legalizes multi-wait
    # instructions via event semaphores, which walrus codegen requires.
    nc = bacc.Bacc("TRN2", target_bir_lowering=False, debug=False,
                   num_devices=N_CORES)
    X = nc.dram_tensor("X", [NB, L, D], f32, kind="ExternalInput")
    cand = nc.dram_tensor("cand", [NB, NCC], i32, kind="ExternalInput")
    aw = nc.dram_tensor("aw", [D, OS], bf16, kind="ExternalInput")
    hw = nc.dram_tensor("hw", [D, LAB], bf16, kind="ExternalInput")
    hb = nc.dram_tensor("hb", [LAB], bf16, kind="ExternalInput")
    lab = nc.dram_tensor("lab", [CLS, LAB], bf16, kind="ExternalInput")
    out_d = nc.dram_tensor("out", [NB, NCC, LAB], f32, kind="ExternalOutput")

    with tile.TileContext(nc) as tc:
        with (
            tc.tile_pool(name="singles", bufs=1) as singles,
            tc.tile_pool(name="gat", bufs=NB * CC) as gat,
            tc.tile_pool(name="xin", bufs=6) as xin,
            tc.tile_pool(name="work", bufs=2) as work,
            tc.tile_pool(name="outp", bufs=4) as outp,
            tc.tile_pool(name="pa", bufs=2, space="PSUM") as pa,
            tc.tile_pool(name="pb", bufs=3, space="PSUM") as pb,
            tc.tile_pool(name="pacc", bufs=1, space="PSUM") as pacc,
        ):
            # ---- constants / weights ----
            ident = singles.tile([P, P], bf16)
            make_identity(nc, ident[:])
            ones_col = singles.tile([P, 1], bf16)
            nc.vector.memset(ones_col[:], 1.0)
            ones_row = singles.tile([1, OS], bf16)
            nc.vector.memset(ones_row[:], 1.0)
            hb_bf = singles.tile([1, LAB], bf16)
            nc.sync.dma_start(out=hb_bf[:], in_=hb[None, :])

            aw_bf = singles.tile([P, DC, OS], bf16)
            nc.sync.dma_start(
                out=aw_bf[:], in_=aw[:, :].rearrange("(c p) o -> p c o", p=P)
            )
            hw_bf = singles.tile([P, DC, LAB], bf16)
            nc.sync.dma_start(
                out=hw_bf[:], in_=hw[:, :].rearrange("(c p) h -> p c h", p=P)
            )
            cand_sb = singles.tile([P, NB, CC], i32)
            nc.gpsimd.dma_start(
                out=cand_sb[:], in_=cand[:, :].rearrange("b (c p) -> p b c", p=P)
            )
            blv_f = {}

            for b in range(NB):
                # ======== phase A: attention pooling over L ========
                xpu = pacc.tile([OS, D], f32, tag="xpu")    # unnormalized alpha^T X
                zz = pacc.tile([OS, 1], f32, tag="zz")      # softmax partition func

                for ii in range(NT // 2):
                    # two l-tiles per DMA halves SWDGE descriptor-gen cost
                    xbf = xin.tile([P, 2, D], bf16, tag="xbf")
                    nc.gpsimd.dma_start(
                        out=xbf[:],
                        in_=X[b, ii * 2 * P:(ii + 1) * 2 * P, :].rearrange(
                            "(t p) d -> p t d", p=P
                        ),
                    )
                    for t in range(2):
                        i = ii * 2 + t
                        # X tile transposed (d on partitions) for the S matmul.
                        # All 8 chunk transposes land in one PSUM bank,
                        # evacuated with a single copy (per-chunk copies
                        # serialize PE against DVE)
                        xt_sb = work.tile([P, DC, P], bf16, tag="xt")
                        tp = pa.tile([P, DC, P], bf16, tag="tp")
                        for c in range(DC):
                            nc.tensor.transpose(
                                out=tp[:, c, :],
                                in_=xbf[:, t, c * P:(c + 1) * P],
                                identity=ident[:],
                            )
                        if i % 3 == 2:
                            nc.scalar.copy(out=xt_sb[:], in_=tp[:])
                        else:
                            nc.vector.tensor_copy(out=xt_sb[:], in_=tp[:])
                        s_ps = pb.tile([P, OS], f32, tag="mm")
                        for c in range(DC):
                            nc.tensor.matmul(
                                out=s_ps[:], lhsT=xt_sb[:, c, :],
                                rhs=aw_bf[:, c, :],
                                start=(c == 0), stop=(c == DC - 1),
                            )
                        e_sb = work.tile([P, OS], bf16, tag="e")
                        nc.scalar.activation(
                            out=e_sb[:], in_=s_ps[:], func=AF.Exp
                        )
                        for nh in range(2):
                            nc.tensor.matmul(
                                out=xpu[:, nh * 512:(nh + 1) * 512],
                                lhsT=e_sb[:],
                                rhs=xbf[:, t, nh * 512:(nh + 1) * 512],
                                start=(i == 0), stop=(i == NT - 1),
                                skip_group_check=True,
                            )
                        nc.tensor.matmul(
                            out=zz[:], lhsT=e_sb[:], rhs=ones_col[:],
                            start=(i == 0), stop=(i == NT - 1),
                            skip_group_check=True,
                        )

                if b == 0:
                    # candidate-row gathers for both batches, issued after
                    # batch 0's X loads so they don't block the SWDGE queue
                    # head (the indirect DMA also tolerates few sync waits,
                    # so each gets a dedicated never-reused slot)
                    for gb in range(NB):
                        for cc in range(CC):
                            bf_t = gat.tile([P, LAB], bf16, tag="blvf",
                                            name=f"blvf_{gb}_{cc}")
                            nc.gpsimd.indirect_dma_start(
                                out=bf_t[:], out_offset=None, in_=lab[:, :],
                                in_offset=bass.IndirectOffsetOnAxis(
                                    ap=cand_sb[:, gb, cc:cc + 1], axis=0,
                                ),
                            )
                            blv_f[gb, cc] = bf_t

                # ======== phase B: normalize + project ========
                rz = work.tile([OS, 1], f32, tag="rz")
                nc.vector.reciprocal(out=rz[:], in_=zz[:])
                xp_bf = work.tile([OS, D], bf16, tag="xp")
                nc.vector.tensor_scalar(
                    out=xp_bf[:], in0=xpu[:], scalar1=rz[:], scalar2=None,
                    op0=mybir.AluOpType.mult,
                )
                # Xp^T (d on partitions) for the h-projection
                xpt_sb = work.tile([P, DC, OS], bf16, tag="xpt")
                tp2 = pa.tile([P, DC, OS], bf16, tag="tp")
                for c in range(DC):
                    nc.tensor.transpose(
                        out=tp2[:, c, :], in_=xp_bf[:, c * P:(c + 1) * P],
                        identity=ident[:OS, :OS],
                    )
                nc.scalar.copy(out=xpt_sb[:], in_=tp2[:])
                # Xpf = relu(Xp @ hw + hb) in natural [OS, LAB] layout; the
                # hb bias rides the PSUM accumulation as a rank-1 matmul
                xpf_sb = work.tile([OS, LAB], bf16, tag="xpf")
                for nh in range(2):
                    xph = pb.tile([OS, 512], f32, tag="mm")
                    for c in range(DC):
                        nc.tensor.matmul(
                            out=xph[:], lhsT=xpt_sb[:, c, :],
                            rhs=hw_bf[:, c, nh * 512:(nh + 1) * 512],
                            start=(c == 0), stop=False,
                        )
                    nc.tensor.matmul(
                        out=xph[:], lhsT=ones_row[:],
                        rhs=hb_bf[:, nh * 512:(nh + 1) * 512],
                        start=False, stop=True,
                    )
                    nc.scalar.activation(
                        out=xpf_sb[:, nh * 512:(nh + 1) * 512], in_=xph[:],
                        func=AF.Relu,
                    )
                # Xpf^T (h on partitions) for the candidate scores
                xpft_sb = work.tile([P, HC, OS], bf16, tag="xpft")
                tp3 = pa.tile([P, HC, OS], bf16, tag="tp")
                for hc in range(HC):
                    nc.tensor.transpose(
                        out=tp3[:, hc, :], in_=xpf_sb[:, hc * P:(hc + 1) * P],
                        identity=ident[:OS, :OS],
                    )
                nc.vector.tensor_copy(out=xpft_sb[:], in_=tp3[:])

                # ======== phase C: candidate gather + attention ========
                blvT = work.tile([P, HC, NCC], bf16, tag="blvT")
                for cc in range(CC):
                    tp4 = pa.tile([P, HC, P], bf16, tag="tp")
                    for hc in range(HC):
                        nc.tensor.transpose(
                            out=tp4[:, hc, :],
                            in_=blv_f[b, cc][:, hc * P:(hc + 1) * P],
                            identity=ident[:],
                        )
                    nc.vector.tensor_copy(
                        out=blvT[:, :, cc * P:(cc + 1) * P], in_=tp4[:]
                    )
                a2t_sb = work.tile([OS, CC, P], bf16, tag="a2t")
                for cc in range(CC):
                    s2 = pb.tile([P, OS], f32, tag="mm")
                    for hc in range(HC):
                        nc.tensor.matmul(
                            out=s2[:],
                            lhsT=blvT[:, hc, cc * P:(cc + 1) * P],
                            rhs=xpft_sb[:, hc, :],
                            start=(hc == 0), stop=(hc == HC - 1),
                        )
                    negm = work.tile([P, 1], f32, tag="negm")
                    nc.vector.tensor_reduce(
                        out=negm[:], in_=s2[:], axis=mybir.AxisListType.X,
                        op=mybir.AluOpType.max, negate=True,
                    )
                    e2 = work.tile([P, OS], bf16, tag="e2")
                    sume = work.tile([P, 1], f32, tag="sume")
                    nc.scalar.activation(
                        out=e2[:], in_=s2[:], func=AF.Exp, bias=negm[:],
                        accum_out=sume[:],
                    )
                    rz2 = work.tile([P, 1], f32, tag="rz2")
                    nc.vector.reciprocal(out=rz2[:], in_=sume[:])
                    a2 = work.tile([P, OS], bf16, tag="a2")
                    nc.vector.tensor_scalar(
                        out=a2[:], in0=e2[:], scalar1=rz2[:], scalar2=None,
                        op0=mybir.AluOpType.mult,
                    )
                    tp5 = pa.tile([OS, P], bf16, tag="tp")
                    nc.tensor.transpose(out=tp5[:], in_=a2[:], identity=ident[:])
                    nc.vector.tensor_copy(out=a2t_sb[:, cc, :], in_=tp5[:])

                # ======== phase D: out = a2^T Xpf ========
                for cc in range(CC):
                    for nh in range(2):
                        op = pb.tile([P, 512], f32, tag="mm")
                        nc.tensor.matmul(
                            out=op[:], lhsT=a2t_sb[:, cc, :],
                            rhs=xpf_sb[:, nh * 512:(nh + 1) * 512],
                            start=True, stop=True,
                        )
                        ob = outp.tile([P, 512], f32, tag="ob")
                        if nh == 0:
                            nc.scalar.copy(out=ob[:], in_=op[:])
                        else:
                            nc.vector.tensor_copy(out=ob[:], in_=op[:])
                        nc.sync.dma_start(
                            out=out_d[b, cc * P:(cc + 1) * P,
                                      nh * 512:(nh + 1) * 512],
                            in_=ob[:],
                        )
    nc.finalize()
    return nc


def _get_program():
    global _PROG
    if _PROG is None:
        _PROG = _build_program()
    return _PROG


def _make_in_maps(inputs):
    import ml_dtypes

    bf16 = ml_dtypes.bfloat16
    X = np.ascontiguousarray(np.asarray(inputs["X"], dtype=np.float32))
    cand = np.ascontiguousarray(
        np.asarray(inputs["candidate"]).astype(np.int32)
    )
    a_w = np.asarray(inputs["a_w"], dtype=np.float32)
    h_w = np.asarray(inputs["h_w"], dtype=np.float32)
    h_b = np.asarray(inputs["h_b"], dtype=np.float32)
    g = np.asarray(inputs["bn_gamma"], dtype=np.float32)
    be = np.asarray(inputs["bn_beta"], dtype=np.float32)
    mu = np.asarray(inputs["bn_mean"], dtype=np.float32)
    var = np.asarray(inputs["bn_var"], dtype=np.float32)
    lab = np.ascontiguousarray(
        np.asarray(inputs["labDescVec"], dtype=np.float32).astype(bf16)
    )

    s = g / np.sqrt(var + BN_EPS)
    hw_eff = np.ascontiguousarray((h_w * s[None, :]).astype(bf16))
    hb_eff = ((h_b - mu) * s + be).astype(bf16)
    aw_bf = a_w.astype(bf16)

    in_maps = []
    for ci in range(N_CORES):
        in_maps.append({
            "X": X[ci * NB:(ci + 1) * NB],
            "cand": cand[ci * NB:(ci + 1) * NB],
            "aw": aw_bf,
            "hw": hw_eff,
            "hb": hb_eff,
            "lab": lab,
        })
    return in_maps


def run(inputs, trace=False, tmpdir=None):
    from concourse.bass_utils import run_bass_kernel_spmd

    nc = _get_program()
    in_maps = _make_in_maps(inputs)
    kwargs = {}
    if trace and tmpdir is None:
        tmpdir = "/root/problem/trace_out"
        import os
        import shutil

        shutil.rmtree(tmpdir, ignore_errors=True)
        os.makedirs(tmpdir, exist_ok=True)
    if tmpdir is not None:
        kwargs["tmpdir"] = tmpdir
    res = run_bass_kernel_spmd(
        nc, in_maps, list(range(N_CORES)), trace=trace, **kwargs,
    )
    out = np.concatenate([r["out"] for r in res.results], axis=0)
    return out, res


def kernel(**inputs):
    out, _ = run(inputs, trace=False)
    return out


# revision 29
# speedup vs baseline: 1.6709x; 1.0181x over previous
"""Trainium2 Bass kernel for DeepICD candidate attention.

Reference computation (per batch b):
    S     = X[b] @ a_w                      [L, OS]     (a_b drops out of softmax)
    alpha = softmax(S, axis=L)
    Xp    = alpha^T @ X[b]                  [OS, D]
    Xph   = Xp @ hw_eff + hb_eff            [OS, LAB]   (BN folded into hw/hb on host)
    Xpf   = relu(Xph)
    bLV   = labDescVec[candidate[b]]        [NC, LAB]
    sc    = Xpf @ bLV^T                     [OS, NC]
    a2    = softmax(sc, axis=OS)
    out   = a2^T @ Xpf                      [NC, LAB]

Sharding: data-parallel over batch B=16 across 8 NeuronCores (2 batches/core);
weights and labDescVec replicated.

softmax over L is computed without max subtraction (S ~ N(0,1), |S| < ~6, exp
is safe in fp32) so the L-dim reduction becomes a matmul with a ones vector.
"""

import numpy as np

P = 128
NB = 2          # batches per core
L = 2048
D = 1024
OS = 64
NCC = 256       # candidates per sample
LAB = 1024
CLS = 8921
NT = L // P     # 16 l-tiles
DC = D // P     # 8 d-chunks
HC = LAB // P   # 8 h-chunks
CC = NCC // P   # 2 candidate chunks
N_CORES = 8
BN_EPS = 1e-5

_PROG = None


def _build_program():
    import concourse.bass as bass
    import concourse.bacc as bacc
    import concourse.tile as tile
    from concourse import mybir
    from concourse.masks import make_identity

    f32 = mybir.dt.float32
    bf16 = mybir.dt.bfloat16
    i32 = mybir.dt.int32
    AF = mybir.ActivationFunctionType

    # Bacc (not plain Bass): its compile() pass legalizes multi-wait
    # instructions via event semaphores, which walrus codegen requires.
    nc = bacc.Bacc("TRN2", target_bir_lowering=False, debug=False,
                   num_devices=N_CORES)
    X = nc.dram_tensor("X", [NB, L, D], f32, kind="ExternalInput")
    cand = nc.dram_tensor("cand", [NB, NCC], i32, kind="ExternalInput")
    aw = nc.dram_tensor("aw", [D, OS], bf16, kind="ExternalInput")
    hw = nc.dram_tensor("hw", [D, LAB], bf16, kind="ExternalInput")
    hb = nc.dram_tensor("hb", [LAB], bf16, kind="ExternalInput")
    lab = nc.dram_tensor("lab", [CLS, LAB], bf16, kind="ExternalInput")
    out_d = nc.dram_tensor("out", [NB, NCC, LAB], f32, kind="ExternalOutput")

    with tile.TileContext(nc) as tc:
        with (
            tc.tile_pool(name="singles", bufs=1) as singles,
            tc.tile_pool(name="gat", bufs=NB * CC) as gat,
            tc.tile_pool(name="xin", bufs=6) as xin,
            tc.tile_pool(name="work", bufs=2) as work,
            tc.tile_pool(name="outp", bufs=4) as outp,
            tc.tile_pool(name="pa", bufs=2, space="PSUM") as pa,
            tc.tile_pool(name="pb", bufs=3, space="PSUM") as pb,
            tc.tile_pool(name="pacc", bufs=1, space="PSUM") as pacc,
        ):
            # ---- constants / weights ----
            ident = singles.tile([P, P], bf16)
            make_identity(nc, ident[:])
            ones_col = singles.tile([P, 1], bf16)
            nc.vector.memset(ones_col[:], 1.0)
            ones_row = singles.tile([1, OS], bf16)
            nc.vector.memset(ones_row[:], 1.0)
            hb_bf = singles.tile([1, LAB], bf16)
            nc.sync.dma_start(out=hb_bf[:], in_=hb[None, :])

            aw_bf = singles.tile([P, DC, OS], bf16)
            nc.sync.dma_start(
                out=aw_bf[:], in_=aw[:, :].rearrange("(c p) o -> p c o", p=P)
            )
            hw_bf = singles.tile([P, DC, LAB], bf16)
            cand_sb = singles.tile([P, NB, CC], i32)
            nc.gpsimd.dma_start(
                out=cand_sb[:], in_=cand[:, :].rearrange("b (c p) -> p b c", p=P)
            )
            blv_f = {}

            for b in range(NB):
                # ======== phase A: attention pooling over L ========
                xpu = pacc.tile([OS, D], f32, tag="xpu")    # unnormalized alpha^T X
                zz = pacc.tile([OS, 1], f32, tag="zz")      # softmax partition func

                for ii in range(NT // 2):
                    # two l-tiles per DMA halves SWDGE descriptor-gen cost
                    xbf = xin.tile([P, 2, D], bf16, tag="xbf")
                    nc.gpsimd.dma_start(
                        out=xbf[:],
                        in_=X[b, ii * 2 * P:(ii + 1) * 2 * P, :].rearrange(
                            "(t p) d -> p t d", p=P
                        ),
                    )
                    for t in range(2):
                        i = ii * 2 + t
                        # X tile transposed (d on partitions) for the S matmul.
                        # All 8 chunk transposes land in one PSUM bank,
                        # evacuated with a single copy (per-chunk copies
                        # serialize PE against DVE)
                        xt_sb = work.tile([P, DC, P], bf16, tag="xt")
                        tp = pa.tile([P, DC, P], bf16, tag="tp")
                        for c in range(DC):
                            nc.tensor.transpose(
                                out=tp[:, c, :],
                                in_=xbf[:, t, c * P:(c + 1) * P],
                                identity=ident[:],
                            )
                        nc.vector.tensor_copy(out=xt_sb[:], in_=tp[:])
                        s_ps = pb.tile([P, OS], f32, tag="mm")
                        for c in range(DC):
                            nc.tensor.matmul(
                                out=s_ps[:], lhsT=xt_sb[:, c, :],
                                rhs=aw_bf[:, c, :],
                                start=(c == 0), stop=(c == DC - 1),
                            )
                        e_sb = work.tile([P, OS], bf16, tag="e")
                        nc.scalar.activation(
                            out=e_sb[:], in_=s_ps[:], func=AF.Exp
                        )
                        for nh in range(2):
                            nc.tensor.matmul(
                                out=xpu[:, nh * 512:(nh + 1) * 512],
                                lhsT=e_sb[:],
                                rhs=xbf[:, t, nh * 512:(nh + 1) * 512],
                                start=(i == 0), stop=(i == NT - 1),
                                skip_group_check=True,
                            )
                        nc.tensor.matmul(
                            out=zz[:], lhsT=e_sb[:], rhs=ones_col[:],
                            start=(i == 0), stop=(i == NT - 1),
                            skip_group_check=True,
                        )

                if b == 0:
                    # bulk weights and candidate-row gathers are issued after
                    # batch 0's X loads so they don't block the DMA stream
                    # head (the indirect DMA also tolerates few sync waits,
                    # so each gets a dedicated never-reused slot)
                    nc.sync.dma_start(
                        out=hw_bf[:],
                        in_=hw[:, :].rearrange("(c p) h -> p c h", p=P),
                    )
                    for gb in range(NB):
                        for cc in range(CC):
                            bf_t = gat.tile([P, LAB], bf16, tag="blvf",
                                            name=f"blvf_{gb}_{cc}")
                            nc.gpsimd.indirect_dma_start(
                                out=bf_t[:], out_offset=None, in_=lab[:, :],
                                in_offset=bass.IndirectOffsetOnAxis(
                                    ap=cand_sb[:, gb, cc:cc + 1], axis=0,
                                ),
                            )
                            blv_f[gb, cc] = bf_t

                # ======== phase B: normalize + project ========
                rz = work.tile([OS, 1], f32, tag="rz")
                nc.vector.reciprocal(out=rz[:], in_=zz[:])
                xp_bf = work.tile([OS, D], bf16, tag="xp")
                nc.vector.tensor_scalar(
                    out=xp_bf[:], in0=xpu[:], scalar1=rz[:], scalar2=None,
                    op0=mybir.AluOpType.mult,
                )
                # Xp^T (d on partitions) for the h-projection
                xpt_sb = work.tile([P, DC, OS], bf16, tag="xpt")
                tp2 = pa.tile([P, DC, OS], bf16, tag="tp")
                for c in range(DC):
                    nc.tensor.transpose(
                        out=tp2[:, c, :], in_=xp_bf[:, c * P:(c + 1) * P],
                        identity=ident[:OS, :OS],
                    )
                nc.scalar.copy(out=xpt_sb[:], in_=tp2[:])
                # Xpf = relu(Xp @ hw + hb) in natural [OS, LAB] layout; the
                # hb bias rides the PSUM accumulation as a rank-1 matmul
                xpf_sb = work.tile([OS, LAB], bf16, tag="xpf")
                for nh in range(2):
                    xph = pb.tile([OS, 512], f32, tag="mm")
                    for c in range(DC):
                        nc.tensor.matmul(
                            out=xph[:], lhsT=xpt_sb[:, c, :],
                            rhs=hw_bf[:, c, nh * 512:(nh + 1) * 512],
                            start=(c == 0), stop=False,
                        )
                    nc.tensor.matmul(
                        out=xph[:], lhsT=ones_row[:],
                        rhs=hb_bf[:, nh * 512:(nh + 1) * 512],
                        start=False, stop=True,
                    )
                    nc.scalar.activation(
                        out=xpf_sb[:, nh * 512:(nh + 1) * 512], in_=xph[:],
                        func=AF.Relu,
                    )
                # Xpf^T (h on partitions) for the candidate scores
                xpft_sb = work.tile([P, HC, OS], bf16, tag="xpft")
                tp3 = pa.tile([P, HC, OS], bf16, tag="tp")
                for hc in range(HC):
                    nc.tensor.transpose(
                        out=tp3[:, hc, :], in_=xpf_sb[:, hc * P:(hc + 1) * P],
                        identity=ident[:OS, :OS],
                    )
                nc.vector.tensor_copy(out=xpft_sb[:], in_=tp3[:])

                # ======== phase C: candidate gather + attention ========
                blvT = work.tile([P, HC, NCC], bf16, tag="blvT")
                for cc in range(CC):
                    tp4 = pa.tile([P, HC, P], bf16, tag="tp")
                    for hc in range(HC):
                        nc.tensor.transpose(
                            out=tp4[:, hc, :],
                            in_=blv_f[b, cc][:, hc * P:(hc + 1) * P],
                            identity=ident[:],
                        )
                    nc.vector.tensor_copy(
                        out=blvT[:, :, cc * P:(cc + 1) * P], in_=tp4[:]
                    )
                # softmax normalization is deferred: out_unnorm = E2^T Xpf,
                # then the PSUM evacuation multiplies by 1/rowsum (the sum is
                # per out-partition, so it rides the evac as a scalar op)
                e2t_sb = work.tile([OS, CC, P], bf16, tag="a2t")
                rz2s = []
                for cc in range(CC):
                    s2 = pb.tile([P, OS], f32, tag="mm")
                    for hc in range(HC):
                        nc.tensor.matmul(
                            out=s2[:],
                            lhsT=blvT[:, hc, cc * P:(cc + 1) * P],
                            rhs=xpft_sb[:, hc, :],
                            start=(hc == 0), stop=(hc == HC - 1),
                        )
                    negm = work.tile([P, 1], f32, tag="negm")
                    nc.vector.tensor_reduce(
                        out=negm[:], in_=s2[:], axis=mybir.AxisListType.X,
                        op=mybir.AluOpType.max, negate=True,
                    )
                    e2 = work.tile([P, OS], bf16, tag="e2")
                    sume = work.tile([P, 1], f32, tag="sume")
                    nc.scalar.activation(
                        out=e2[:], in_=s2[:], func=AF.Exp, bias=negm[:],
                        accum_out=sume[:],
                    )
                    rz2 = work.tile([P, 1], f32, tag="rz2", name=f"rz2_{b}_{cc}")
                    nc.vector.reciprocal(out=rz2[:], in_=sume[:])
                    rz2s.append(rz2)
                    tp5 = pa.tile([OS, P], bf16, tag="tp")
                    nc.tensor.transpose(out=tp5[:], in_=e2[:], identity=ident[:])
                    nc.vector.tensor_copy(out=e2t_sb[:, cc, :], in_=tp5[:])

                # ======== phase D: out = softmax(s2)^T Xpf ========
                for cc in range(CC):
                    for nh in range(2):
                        op = pb.tile([P, 512], f32, tag="mm")
                        nc.tensor.matmul(
                            out=op[:], lhsT=e2t_sb[:, cc, :],
                            rhs=xpf_sb[:, nh * 512:(nh + 1) * 512],
                            start=True, stop=True,
                        )
                        ob = outp.tile([P, 512], f32, tag="ob")
                        if nh == 0:
                            nc.scalar.activation(
                                out=ob[:], in_=op[:], func=AF.Copy,
                                scale=rz2s[cc][:],
                            )
                        else:
                            nc.vector.tensor_scalar(
                                out=ob[:], in0=op[:], scalar1=rz2s[cc][:],
                                scalar2=None, op0=mybir.AluOpType.mult,
                            )
                        nc.sync.dma_start(
                            out=out_d[b, cc * P:(cc + 1) * P,
                                      nh * 512:(nh + 1) * 512],
                            in_=ob[:],
                        )
    nc.finalize()
    return nc


def _get_program():
    global _PROG
    if _PROG is None:
        _PROG = _build_program()
    return _PROG


def _make_in_maps(inputs):
    import ml_dtypes

    bf16 = ml_dtypes.bfloat16
    X = np.ascontiguousarray(np.asarray(inputs["X"], dtype=np.float32))
    cand = np.ascontiguousarray(
        np.asarray(inputs["candidate"]).astype(np.int32)
    )
    a_w = np.asarray(inputs["a_w"], dtype=np.float32)
    h_w = np.asarray(inputs["h_w"], dtype=np.float32)
    h_b = np.asarray(inputs["h_b"], dtype=np.float32)
    g = np.asarray(inputs["bn_gamma"], dtype=np.float32)
    be = np.asarray(inputs["bn_beta"], dtype=np.float32)
    mu = np.asarray(inputs["bn_mean"], dtype=np.float32)
    var = np.asarray(inputs["bn_var"], dtype=np.float32)
    lab = np.ascontiguousarray(
        np.asarray(inputs["labDescVec"], dtype=np.float32).astype(bf16)
    )

    s = g / np.sqrt(var + BN_EPS)
    hw_eff = np.ascontiguousarray((h_w * s[None, :]).astype(bf16))
    hb_eff = ((h_b - mu) * s + be).astype(bf16)
    aw_bf = a_w.astype(bf16)

    in_maps = []
    for ci in range(N_CORES):
        in_maps.append({
            "X": X[ci * NB:(ci + 1) * NB],
            "cand": cand[ci * NB:(ci + 1) * NB],
            "aw": aw_bf,
            "hw": hw_eff,
            "hb": hb_eff,
            "lab": lab,
        })
    return in_maps


def run(inputs, trace=False, tmpdir=None):
    from concourse.bass_utils import run_bass_kernel_spmd

    nc = _get_program()
    in_maps = _make_in_maps(inputs)
    kwargs = {}
    if trace and tmpdir is None:
        tmpdir = "/root/problem/trace_out"
        import os
        import shutil

        shutil.rmtree(tmpdir, ignore_errors=True)
        os.makedirs(tmpdir, exist_ok=True)
    if tmpdir is not None:
        kwargs["tmpdir"] = tmpdir
    res = run_bass_kernel_spmd(
        nc, in_maps, list(range(N_CORES)), trace=trace, **kwargs,
    )
    out = np.concatenate([r["out"] for r in res.results], axis=0)
    return out, res


def kernel(**inputs):
    out, _ = run(inputs, trace=False)
    return out
